# revision 15
# baseline (speedup 1.0000x reference)
"""Causal self-attention (B=2, S=2048, E=2048, H=16, rope) on 8 TRN2 NeuronCores.

Sharding: tensor-parallel over heads. Each core owns 2 heads (both batches):
w_qkv rows / w_out columns for its heads; every core reads the full x
(replicated, bf16, pre-transposed) and produces a partial [B*S, E] bf16
output; the host sums the 8 partials (the "all-reduce").

Per-core kernel (v2 — pipelined):
  - xT [E, B*S] bf16 serves as matmul rhs (Q/K projections -> QT/KT arrive
    transposed [D, S], the layout attention wants) and as lhsT (V
    projection, natural [S, D]).
  - scores are computed transposed: scoresT[k,q] = KT^T @ QT, in panels of
    512 q columns. The attention inner loop is software-pipelined for the
    in-order PE queue: scores(kb+1) is emitted BEFORE A@V(kb), so the PE
    never waits on ScalarE's exp.
  - softmax sums over k no longer burn a per-kb PE matmul: DVE accumulates
    the exp'd tiles into a per-panel f32 acc, and ONE ones[128,128] matmul
    per panel produces the column sums broadcast across partitions;
    reciprocal + multiply fold normalization into the y^T PSUM evacuation.
  - out_proj for panel p is deferred and dribbled into the (scalar-bound)
    attention stages of panel p+1 as PE filler work; its PSUM tiles share
    a bank ring with the per-panel sums tiles.
  - rope is applied on DVE during QKV-PSUM evacuation with [D, S] cos /
    signed-sin tables, DMA'd per 512-token block on the fast HWDGE queue.
  - output partials are bf16 (halves the output DMA; host sums in f32).
"""

import math

import numpy as np
import ml_dtypes

import concourse.bass as bass
import concourse.mybir as mybir
import concourse.tile as tile
from concourse import bacc
from concourse.bass_utils import run_bass_kernel_spmd

B, S, E, H, D = 2, 2048, 2048, 16, 128
NCORES = 8
HL = H // NCORES            # heads per core = 2
NTOK = B * S                # 4096
KE = E // 128               # 16 contraction chunks
NB = S // 128               # 16 k/token blocks per batch
NPANEL = S // 512           # 4 q panels per batch
SOFTMAX_SCALE = 1.0 / math.sqrt(D)
BF16 = mybir.dt.bfloat16
F32 = mybir.dt.float32

ROPE_BASE = 10000.0


def _rope_tables():
    inv_freq = 1.0 / (ROPE_BASE ** (np.arange(0, D, 2, dtype=np.float32) / D))
    pos = np.arange(S, dtype=np.float32)
    freqs = np.outer(pos, inv_freq)               # [S, D/2]
    emb = np.concatenate([freqs, freqs], -1)      # [S, D]
    cosT = np.cos(emb).T.astype(np.float32)       # [D, S]
    sinT = np.sin(emb).T.astype(np.float32)
    sinS = sinT.copy()
    sinS[: D // 2] *= -1.0                        # signed: rotate_half sign folded in
    return np.ascontiguousarray(cosT), np.ascontiguousarray(sinS)


def _emit(nc, tc, xT, wqkvT, w_outT, out, cos_d, sin_d, mask_d):
    from contextlib import ExitStack

    ctx = ExitStack()
    with ctx:
        singles = ctx.enter_context(tc.tile_pool(name="singles", bufs=1))
        xpool = ctx.enter_context(tc.tile_pool(name="xcol", bufs=2))
        persist = ctx.enter_context(tc.tile_pool(name="persist", bufs=1))
        ropet = ctx.enter_context(tc.tile_pool(name="ropet", bufs=3))
        attnp = ctx.enter_context(tc.tile_pool(name="attn", bufs=8))
        accp = ctx.enter_context(tc.tile_pool(name="accp", bufs=2))
        evacp = ctx.enter_context(tc.tile_pool(name="evac", bufs=2))
        outp = ctx.enter_context(tc.tile_pool(name="outp", bufs=4))
        psum = ctx.enter_context(tc.tile_pool(name="psum", bufs=2, space="PSUM"))

        # ---- constant tiles (bulky low-priority ones ride the slow SWDGE
        # queues; cos/sin slices ride the fast HWDGE queue per block) ----
        wq_sb = [singles.tile([128, 3 * HL * D], BF16, tag=f"wq{ke}", name=f"wq{ke}")
                 for ke in range(KE)]
        wo_sb = singles.tile([128, HL, E], BF16, tag="wo")
        cos_sb = singles.tile([128, S], BF16, tag="cos")
        sin_sb = singles.tile([128, S], BF16, tag="sin")
        mask_sb = singles.tile([128, 128], BF16, tag="mask")
        ones_kk = singles.tile([128, 128], BF16, tag="oneskk")
        nc.vector.memset(ones_kk, 1.0)
        for hl in range(HL):
            nc.gpsimd.dma_start(
                out=wo_sb[:, hl, :], in_=w_outT[hl * 128:(hl + 1) * 128, :]
            )
        nc.gpsimd.dma_start(out=mask_sb, in_=mask_d)

        # ---- persistent per-(b,h) tensors ----
        q_sb = [[persist.tile([128, S], BF16, tag=f"q{b}{h}", name=f"q{b}{h}") for h in range(HL)] for b in range(B)]
        k_sb = [[persist.tile([128, S], BF16, tag=f"k{b}{h}", name=f"k{b}{h}") for h in range(HL)] for b in range(B)]
        v_sb = [persist.tile([128, NB, HL * D], BF16, tag=f"v{b}", name=f"v{b}") for b in range(B)]
        y_sb = [[persist.tile([128, S], BF16, tag=f"y{b}{h}", name=f"y{b}{h}") for h in range(HL)] for b in range(B)]

        # ---- PE filler queue: deferred out_proj tile units dribbled into
        # attention stages / projection waves to fill PE idle windows ----
        filler = []

        def pump(n):
            for _ in range(min(n, len(filler))):
                filler.pop(0)()

        def outproj_unit(b, tkb, oc):
            def emit():
                tok0 = b * S + tkb * 128
                ops = psum.tile([128, 512], F32, tag="sq", bufs=3,
                                name=f"o{b}{tkb}{oc}")
                for hl in range(HL):
                    nc.tensor.matmul(
                        ops,
                        lhsT=y_sb[b][hl][:, tkb * 128:(tkb + 1) * 128],
                        rhs=wo_sb[:, hl, oc * 512:(oc + 1) * 512],
                        start=(hl == 0),
                        stop=(hl == HL - 1),
                    )
                ot = outp.tile([128, 512], BF16, tag="ot", name=f"ot{b}{tkb}{oc}")
                if (tkb + oc) % 2 == 0:
                    nc.scalar.copy(ot, ops)
                else:
                    nc.vector.tensor_copy(ot, ops)
                nc.sync.dma_start(
                    out=out[tok0:tok0 + 128, oc * 512:(oc + 1) * 512], in_=ot
                )
            return emit

        def queue_outproj(b, p):
            for tkb in range(4 * p, 4 * p + 4):
                for oc in range(E // 512):
                    filler.append(outproj_unit(b, tkb, oc))

        def proj_batch(b):
            for sb4 in range(S // 512):        # 4 column-blocks of 512 tokens
                tb = b * (S // 512) + sb4
                soff = sb4 * 512
                xc = []
                for ke in range(KE):
                    x1 = xpool.tile([128, 512], BF16, tag=f"xc{ke}", name=f"xc{tb}_{ke}")
                    if tb == 0:
                        # interleave weight/x loads so matmul ke starts after
                        # ~2 small DMAs instead of after the whole input load
                        nc.sync.dma_start(
                            out=wq_sb[ke], in_=wqkvT[ke * 128:(ke + 1) * 128, :]
                        )
                    nc.sync.dma_start(
                        out=x1,
                        in_=xT[ke * 128:(ke + 1) * 128, tb * 512:(tb + 1) * 512],
                    )
                    if b == 0 and ke == 3:
                        # rope table slice for this block on the fast queue
                        nc.sync.dma_start(
                            out=cos_sb[:, soff:soff + 512],
                            in_=cos_d[:, soff:soff + 512],
                        )
                        nc.sync.dma_start(
                            out=sin_sb[:, soff:soff + 512],
                            in_=sin_d[:, soff:soff + 512],
                        )
                    xc.append(x1)
                # 8 accumulation chains (4 QK rows + 4 V token-blocks) in
                # waves, interleaved per-ke: the PE is in-order, so within a
                # wave each arriving xc chunk feeds matmuls back to back
                # instead of one chain stalling on the next DMA
                chains = [("qk", rb) for rb in range(2 * HL)] + [
                    ("v", tsb) for tsb in range(4)
                ]
                if tb == 0:
                    # DMA-paced first block: advance chains in pairs per-ke
                    waves = [chains[i:i + 2] for i in range(0, 8, 2)]
                else:
                    waves = [[c] for c in chains]
                for wv, wave in enumerate(waves):
                    pss = [
                        psum.tile([128, 512], F32, tag="ps", bufs=3,
                                  name=f"p{tb}_{wv}{j}")
                        for j in range(len(wave))
                    ]
                    for ke in range(KE):
                        for j, (kind, idx) in enumerate(wave):
                            if kind == "qk":
                                nc.tensor.matmul(
                                    pss[j],
                                    lhsT=wq_sb[ke][:, idx * 128:(idx + 1) * 128],
                                    rhs=xc[ke],
                                    start=(ke == 0),
                                    stop=(ke == KE - 1),
                                )
                            else:
                                nc.tensor.matmul(
                                    pss[j][:, 0:HL * D],
                                    lhsT=xc[ke][:, idx * 128:(idx + 1) * 128],
                                    rhs=wq_sb[ke][:, 2 * HL * 128:],
                                    start=(ke == 0),
                                    stop=(ke == KE - 1),
                                )
                    for j, (kind, idx) in enumerate(wave):
                        ps = pss[j]
                        if kind == "qk":
                            rb = idx
                            # rope: dst = t*cos + swap(t)*sin_signed, bf16 out
                            dst = (q_sb if rb < HL else k_sb)[b][rb % HL]
                            sl = bass.ds(soff, 512)
                            tsw = ropet.tile([128, 512], F32, tag="tsw", name=f"tsw{tb}{rb}")
                            tco = ropet.tile([128, 512], F32, tag="tco", name=f"tco{tb}{rb}")
                            # PSUM reads stay on DVE; the SBUF-only combine
                            # rides the otherwise-idle GpSimd
                            nc.vector.tensor_mul(tsw[0:64, :], ps[64:128, :], sin_sb[0:64, sl])
                            nc.vector.tensor_mul(tsw[64:128, :], ps[0:64, :], sin_sb[64:128, sl])
                            nc.vector.tensor_mul(tco, ps, cos_sb[:, sl])
                            nc.gpsimd.tensor_add(dst[:, sl], tco, tsw)
                        else:
                            blk = (soff // 128) + idx
                            nc.scalar.copy(v_sb[b][:, blk, :], ps[:, 0:HL * D])
                    pump(1)

        def attn_batch(b):
            # flat, software-pipelined stage list: forward panels so the
            # deferred out_proj of panel p fills panel p+1's stages
            flat = []
            for p in range(NPANEL):
                for hl in range(HL):
                    flat.append((hl, p, None))   # panel marker for acc reset
                    for kb in range(4 * p + 4):
                        flat.append((hl, p, kb))

            prev = None          # pending stage awaiting A@V + acc-add
            acc = {}             # (hl,p) -> f32 acc tile
            pend = {}            # (hl,p) -> unpaired (at, qoff)
            yps = {}             # (hl,p) -> psum tile
            state = {}
            pending = []         # panels awaiting finalize, deferred 2 stages
            pair_ct = [0]

            def emit_av(st, idx):
                hl, p, kb = st
                nkb = 4 * p + 4
                qoff = max(0, kb - 4 * p) * 128
                at = state[(hl, p, kb)]
                nc.tensor.matmul(
                    yps[(hl, p)][:, qoff:512],
                    lhsT=v_sb[b][:, kb, hl * D:(hl + 1) * D],
                    rhs=at[:, qoff:512],
                    start=(kb == 0),
                    stop=(kb == nkb - 1),
                )
                # exp'd-tile accumulation (replaces per-kb PE sums matmuls):
                # independent pair-adds split DVE/GpSimd, serial chain on DVE
                key = (hl, p)
                if key not in pend:
                    pend[key] = (at, qoff)
                else:
                    at0, q0 = pend.pop(key)
                    eng = nc.gpsimd if pair_ct[0] % 5 < 3 else nc.vector
                    pair_ct[0] += 1
                    a = acc.get(key)
                    if a is None:
                        t = accp.tile([128, 512], F32, tag="acc", name=f"ac{b}{hl}{p}")
                        acc[key] = t
                    else:
                        t = accp.tile([128, 512], F32, tag="pair", name=f"pr{b}{hl}{p}{kb}")
                    eng.tensor_add(t[:, qoff:512], at0[:, qoff:512], at[:, qoff:512])
                    if qoff > q0:
                        nc.vector.tensor_copy(t[:, q0:qoff], at0[:, q0:qoff])
                    if a is not None:
                        nc.vector.tensor_add(a[:, q0:512], a[:, q0:512], t[:, q0:512])
                del state[(hl, p, kb)]
                if kb == nkb - 1:
                    # emit the bf16 cast now (DVE); defer the PE ones-matmul
                    # finalize ~2 stages so the cast is done when PE gets there
                    a = acc.pop((hl, p))
                    ab = accp.tile([128, 512], BF16, tag="accb", name=f"ab{b}{hl}{p}")
                    nc.gpsimd.tensor_copy(ab, a)
                    pending.append((hl, p, ab, idx))

            def panel_end(hl, p, ab):
                sps = psum.tile([128, 512], F32, tag="sq", bufs=3, name=f"s{b}{hl}{p}")
                nc.tensor.matmul(sps, lhsT=ones_kk, rhs=ab, start=True, stop=True)
                rb_sb = evacp.tile([128, 512], F32, tag="rb", name=f"rb{b}{hl}{p}")
                nc.vector.reciprocal_approx_fast(out=rb_sb, in_=sps)
                nc.vector.tensor_mul(
                    y_sb[b][hl][:, p * 512:(p + 1) * 512], yps.pop((hl, p)), rb_sb
                )
                if hl == HL - 1:
                    queue_outproj(b, p)

            i = 0
            for st in flat:
                hl, p, kb = st
                if kb is None:
                    yps[(hl, p)] = psum.tile([128, 512], F32, tag="yps", bufs=2,
                                             name=f"yps{b}{hl}{p}")
                    continue
                i += 1
                qoff = max(0, kb - 4 * p) * 128
                ps = psum.tile([128, 512], F32, tag="ps", bufs=3, name=f"sc{b}{hl}{p}{kb}")
                nc.tensor.matmul(
                    ps[:, 0:512 - qoff],
                    lhsT=k_sb[b][hl][:, kb * 128:(kb + 1) * 128],
                    rhs=q_sb[b][hl][:, p * 512 + qoff:(p + 1) * 512],
                    start=True,
                    stop=True,
                )
                if pending and pending[0][3] <= i - 2:
                    ehl, ep, eab, _ = pending.pop(0)
                    panel_end(ehl, ep, eab)
                if prev is not None:
                    emit_av(prev, i)
                at = attnp.tile([128, 512], BF16, tag="attn", name=f"at{b}{hl}{p}{kb}")
                nc.scalar.activation(
                    at[:, qoff:512],
                    ps[:, 0:512 - qoff],
                    mybir.ActivationFunctionType.Exp,
                    scale=SOFTMAX_SCALE,
                )
                if kb >= 4 * p:  # diagonal block: zero the k>q half
                    nc.gpsimd.tensor_mul(
                        at[:, qoff:qoff + 128], at[:, qoff:qoff + 128], mask_sb
                    )
                state[(hl, p, kb)] = at
                prev = st
                pump(1)
            emit_av(prev, i + 1)
            while pending:
                ehl, ep, eab, _ = pending.pop(0)
                panel_end(ehl, ep, eab)

        proj_batch(0)
        attn_batch(0)
        proj_batch(1)            # pumps leftover outproj(b0, p3) units
        attn_batch(1)
        pump(len(filler))        # final outproj(b1, p3) tail


def build():
    nc = bacc.Bacc("TRN2", target_bir_lowering=False, debug=False)
    xT = nc.dram_tensor("xT", [E, NTOK], BF16, kind="ExternalInput").ap()
    wqkvT = nc.dram_tensor("wqkvT", [E, 3 * HL * D], BF16, kind="ExternalInput").ap()
    w_outT = nc.dram_tensor("w_outT", [HL * D, E], BF16, kind="ExternalInput").ap()
    out = nc.dram_tensor("out", [NTOK, E], BF16, kind="ExternalOutput").ap()

    cosT, sinS = _rope_tables()
    cos_d = nc.inline_tensor(cosT.astype(ml_dtypes.bfloat16), name="cos_t").ap()
    sin_d = nc.inline_tensor(sinS.astype(ml_dtypes.bfloat16), name="sin_t").ap()
    # maskT01[k, q] = 1 where k <= q (valid), else 0 — transposed-causal
    mask = np.triu(np.ones((128, 128), np.float32)).astype(ml_dtypes.bfloat16)
    mask_d = nc.inline_tensor(mask, name="maskT01").ap()

    with tile.TileContext(nc) as tc:
        _emit(nc, tc, xT, wqkvT, w_outT, out, cos_d, sin_d, mask_d)
    nc.compile()
    return nc


def make_in_maps(x, w_qkv, w_out):
    bf = ml_dtypes.bfloat16
    x2 = np.asarray(x, np.float32).reshape(NTOK, E)
    xT = np.ascontiguousarray(x2.astype(bf).T)                      # [E, NTOK]
    w_qkv = np.asarray(w_qkv, np.float32)
    w_out = np.asarray(w_out, np.float32)
    in_maps = []
    for c in range(NCORES):
        hs = [HL * c + j for j in range(HL)]
        rows = np.concatenate(
            [w_qkv[t * E + h * D:t * E + (h + 1) * D] for t in range(3) for h in hs]
        )                                                           # [768, E]
        wqkvT = np.ascontiguousarray(rows.astype(bf).T)             # [E, 768]
        w_outT = np.ascontiguousarray(
            w_out[:, c * HL * D:(c + 1) * HL * D].astype(bf).T      # [256, E]
        )
        in_maps.append({"xT": xT, "wqkvT": wqkvT, "w_outT": w_outT})
    return in_maps


_NC = None


def kernel(x, w_qkv, w_out):
    global _NC
    if _NC is None:
        _NC = build()
    in_maps = make_in_maps(x, w_qkv, w_out)
    res = run_bass_kernel_spmd(_NC, in_maps, core_ids=list(range(NCORES)))
    total = np.zeros((NTOK, E), np.float32)
    for r in res.results:
        total += r["out"]
    return total.reshape(B, S, E)


# revision 24
# speedup vs baseline: 1.0600x; 1.0600x over previous
"""Causal self-attention (B=2, S=2048, E=2048, H=16, rope) on 8 TRN2 NeuronCores.

Sharding: tensor-parallel over heads. Each core owns 2 heads (both batches):
w_qkv rows / w_out columns for its heads; every core reads the full x
(replicated, bf16, pre-transposed) and produces a partial [B*S, E] bf16
output; the host sums the 8 partials (the "all-reduce").

Per-core kernel (v4 — pipelined):
  - xT [E, B*S] bf16 serves as matmul rhs (Q/K projections -> QT/KT arrive
    transposed [D, S], the layout attention wants) and as lhsT (V
    projection, natural [S, D]).
  - scores are computed transposed: scoresT[k,q] = KT^T @ QT, in panels of
    512 q columns. The attention inner loop is software-pipelined for the
    in-order PE queue: scores(kb+1) is emitted BEFORE A@V(kb)/sums(kb), so
    the PE never waits on ScalarE's exp; panel finalize (reciprocal +
    normalize) is deferred one stage for the same reason.
  - softmax sums over k accumulate on the PE via a ones[128,128] matmul per
    k-block (attention is PE/ScalarE co-paced, so this is free; DVE/GpSimd
    replacements measured slower on HW).
  - out_proj for panel p is deferred and dribbled into the attention stages
    of panel p+1 / the next projection's waves as PE filler work.
  - rope is applied during QKV-PSUM evacuation with bf16 [D, S] cos /
    signed-sin tables DMA'd per 512-token block on the fast HWDGE queue;
    the PSUM-reading muls run on DVE, the SBUF-only combine on GpSimd, and
    V-evacuation on ScalarE to keep DVE off the projection critical path.
  - output partials are bf16 (halves the output DMA; host sums in f32).
"""

import math

import numpy as np
import ml_dtypes

import concourse.bass as bass
import concourse.mybir as mybir
import concourse.tile as tile
from concourse import bacc
from concourse.bass_utils import run_bass_kernel_spmd

B, S, E, H, D = 2, 2048, 2048, 16, 128
NCORES = 8
HL = H // NCORES            # heads per core = 2
NTOK = B * S                # 4096
KE = E // 128               # 16 contraction chunks
NB = S // 128               # 16 k/token blocks per batch
NPANEL = S // 512           # 4 q panels per batch
SOFTMAX_SCALE = 1.0 / math.sqrt(D)
BF16 = mybir.dt.bfloat16
F32 = mybir.dt.float32

ROPE_BASE = 10000.0


def _rope_tables():
    inv_freq = 1.0 / (ROPE_BASE ** (np.arange(0, D, 2, dtype=np.float32) / D))
    pos = np.arange(S, dtype=np.float32)
    freqs = np.outer(pos, inv_freq)               # [S, D/2]
    emb = np.concatenate([freqs, freqs], -1)      # [S, D]
    cosT = np.cos(emb).T.astype(np.float32)       # [D, S]
    sinT = np.sin(emb).T.astype(np.float32)
    sinS = sinT.copy()
    sinS[: D // 2] *= -1.0                        # signed: rotate_half sign folded in
    return np.ascontiguousarray(cosT), np.ascontiguousarray(sinS)


def _emit(nc, tc, xT, wqkvT, w_outT, out, cos_d, sin_d, mask_d):
    from contextlib import ExitStack

    ctx = ExitStack()
    with ctx:
        singles = ctx.enter_context(tc.tile_pool(name="singles", bufs=1))
        xpool = ctx.enter_context(tc.tile_pool(name="xcol", bufs=2))
        persist = ctx.enter_context(tc.tile_pool(name="persist", bufs=1))
        ropet = ctx.enter_context(tc.tile_pool(name="ropet", bufs=3))
        attnp = ctx.enter_context(tc.tile_pool(name="attn", bufs=8))
        accp = ctx.enter_context(tc.tile_pool(name="accp", bufs=2))
        evacp = ctx.enter_context(tc.tile_pool(name="evac", bufs=2))
        outp = ctx.enter_context(tc.tile_pool(name="outp", bufs=4))
        psum = ctx.enter_context(tc.tile_pool(name="psum", bufs=2, space="PSUM"))

        # ---- constant tiles (bulky low-priority ones ride the slow SWDGE
        # queues; cos/sin slices ride the fast HWDGE queue per block) ----
        wq_sb = [singles.tile([128, 3 * HL * D], BF16, tag=f"wq{ke}", name=f"wq{ke}")
                 for ke in range(KE)]
        wo_sb = singles.tile([128, HL, E], BF16, tag="wo")
        cos_sb = singles.tile([128, S], BF16, tag="cos")
        sin_sb = singles.tile([128, S], BF16, tag="sin")
        mask_sb = singles.tile([128, 128], BF16, tag="mask")
        ones_kk = singles.tile([128, 128], BF16, tag="oneskk")
        nc.vector.memset(ones_kk, 1.0)

        def load_consts():
            for hl in range(HL):
                nc.gpsimd.dma_start(
                    out=wo_sb[:, hl, :], in_=w_outT[hl * 128:(hl + 1) * 128, :]
                )
            nc.gpsimd.dma_start(out=mask_sb, in_=mask_d)

        # ---- persistent per-(b,h) tensors ----
        q_sb = [[persist.tile([128, S], BF16, tag=f"q{b}{h}", name=f"q{b}{h}") for h in range(HL)] for b in range(B)]
        k_sb = [[persist.tile([128, S], BF16, tag=f"k{b}{h}", name=f"k{b}{h}") for h in range(HL)] for b in range(B)]
        v_sb = [persist.tile([128, NB, HL * D], BF16, tag=f"v{b}", name=f"v{b}") for b in range(B)]
        y_sb = [[persist.tile([128, S], BF16, tag=f"y{b}{h}", name=f"y{b}{h}") for h in range(HL)] for b in range(B)]

        # ---- PE filler queue: deferred out_proj tile units dribbled into
        # attention stages / projection waves to fill PE idle windows ----
        filler = []

        def pump(n):
            for _ in range(min(n, len(filler))):
                filler.pop(0)()

        def outproj_unit(b, tkb, oc):
            def emit():
                tok0 = b * S + tkb * 128
                ops = psum.tile([128, 512], F32, tag="sq", bufs=3,
                                name=f"o{b}{tkb}{oc}")
                for hl in range(HL):
                    nc.tensor.matmul(
                        ops,
                        lhsT=y_sb[b][hl][:, tkb * 128:(tkb + 1) * 128],
                        rhs=wo_sb[:, hl, oc * 512:(oc + 1) * 512],
                        start=(hl == 0),
                        stop=(hl == HL - 1),
                    )
                ot = outp.tile([128, 512], BF16, tag="ot", name=f"ot{b}{tkb}{oc}")
                nc.vector.tensor_copy(ot, ops)
                nc.sync.dma_start(
                    out=out[tok0:tok0 + 128, oc * 512:(oc + 1) * 512], in_=ot
                )
            return emit

        def queue_outproj(b, p):
            for tkb in range(4 * p, 4 * p + 4):
                for oc in range(E // 512):
                    filler.append(outproj_unit(b, tkb, oc))

        def proj_batch(b):
            for sb4 in range(S // 512):        # 4 column-blocks of 512 tokens
                tb = b * (S // 512) + sb4
                soff = sb4 * 512
                if tb == 2:
                    load_consts()
                xc = []
                for ke in range(KE):
                    x1 = xpool.tile([128, 512], BF16, tag=f"xc{ke}", name=f"xc{tb}_{ke}")
                    if tb == 0:
                        # first waves read only wq cols 0:256 — load that
                        # slice with x; the rest streams behind (below)
                        nc.sync.dma_start(
                            out=wq_sb[ke][:, 0:256],
                            in_=wqkvT[ke * 128:(ke + 1) * 128, 0:256],
                        )
                    nc.sync.dma_start(
                        out=x1,
                        in_=xT[ke * 128:(ke + 1) * 128, tb * 512:(tb + 1) * 512],
                    )
                    if b == 0 and ke == 3:
                        # rope table slice for this block on the fast queue
                        nc.sync.dma_start(
                            out=cos_sb[:, soff:soff + 512],
                            in_=cos_d[:, soff:soff + 512],
                        )
                        nc.sync.dma_start(
                            out=sin_sb[:, soff:soff + 512],
                            in_=sin_d[:, soff:soff + 512],
                        )
                    xc.append(x1)
                if tb == 0:
                    for ke in range(KE):
                        nc.sync.dma_start(
                            out=wq_sb[ke][:, 256:512],
                            in_=wqkvT[ke * 128:(ke + 1) * 128, 256:512],
                        )
                    for ke in range(KE):
                        nc.sync.dma_start(
                            out=wq_sb[ke][:, 512:768],
                            in_=wqkvT[ke * 128:(ke + 1) * 128, 512:768],
                        )
                # 8 accumulation chains (4 QK rows + 4 V token-blocks) in
                # waves, interleaved per-ke: the PE is in-order, so within a
                # wave each arriving xc chunk feeds matmuls back to back
                # instead of one chain stalling on the next DMA
                chains = [("qk", rb) for rb in range(2 * HL)] + [
                    ("v", tsb) for tsb in range(4)
                ]
                if tb == 0:
                    # DMA-paced first block: advance chains in pairs per-ke
                    waves = [chains[i:i + 2] for i in range(0, 8, 2)]
                else:
                    waves = [[c] for c in chains]
                for wv, wave in enumerate(waves):
                    pss = [
                        psum.tile([128, 512], F32, tag="ps", bufs=3,
                                  name=f"p{tb}_{wv}{j}")
                        for j in range(len(wave))
                    ]
                    for ke in range(KE):
                        for j, (kind, idx) in enumerate(wave):
                            if kind == "qk":
                                nc.tensor.matmul(
                                    pss[j],
                                    lhsT=wq_sb[ke][:, idx * 128:(idx + 1) * 128],
                                    rhs=xc[ke],
                                    start=(ke == 0),
                                    stop=(ke == KE - 1),
                                )
                            else:
                                nc.tensor.matmul(
                                    pss[j][:, 0:HL * D],
                                    lhsT=xc[ke][:, idx * 128:(idx + 1) * 128],
                                    rhs=wq_sb[ke][:, 2 * HL * 128:],
                                    start=(ke == 0),
                                    stop=(ke == KE - 1),
                                )
                    for j, (kind, idx) in enumerate(wave):
                        ps = pss[j]
                        if kind == "qk":
                            rb = idx
                            # rope: dst = t*cos + swap(t)*sin_signed, bf16 out
                            dst = (q_sb if rb < HL else k_sb)[b][rb % HL]
                            sl = bass.ds(soff, 512)
                            tsw = ropet.tile([128, 512], F32, tag="tsw", name=f"tsw{tb}{rb}")
                            tco = ropet.tile([128, 512], F32, tag="tco", name=f"tco{tb}{rb}")
                            # PSUM reads stay on DVE; the SBUF-only combine
                            # rides the otherwise-idle GpSimd
                            nc.vector.tensor_mul(tsw[0:64, :], ps[64:128, :], sin_sb[0:64, sl])
                            nc.vector.tensor_mul(tsw[64:128, :], ps[0:64, :], sin_sb[64:128, sl])
                            nc.vector.tensor_mul(tco, ps, cos_sb[:, sl])
                            nc.gpsimd.tensor_add(dst[:, sl], tco, tsw)
                        else:
                            blk = (soff // 128) + idx
                            nc.scalar.copy(v_sb[b][:, blk, :], ps[:, 0:HL * D])
                    pump(1)

        def attn_batch(b):
            # flat, software-pipelined stage list: forward panels so the
            # deferred out_proj of panel p fills panel p+1's stages
            flat = []
            for p in range(NPANEL):
                for hl in range(HL):
                    flat.append((hl, p, None))   # panel marker for acc reset
                    for kb in range(4 * p + 4):
                        flat.append((hl, p, kb))

            prev = None          # pending stage awaiting A@V + sums
            yps = {}             # (hl,p) -> psum tile
            sps = {}             # (hl,p) -> psum sums tile
            state = {}
            pending = []         # panels awaiting finalize, deferred 1 stage

            def emit_av(st, idx):
                hl, p, kb = st
                nkb = 4 * p + 4
                qoff = max(0, kb - 4 * p) * 128
                at = state.pop((hl, p, kb))
                nc.tensor.matmul(
                    yps[(hl, p)][:, qoff:512],
                    lhsT=v_sb[b][:, kb, hl * D:(hl + 1) * D],
                    rhs=at[:, qoff:512],
                    start=(kb == 0),
                    stop=(kb == nkb - 1),
                )
                nc.tensor.matmul(
                    sps[(hl, p)][:, qoff:512],
                    lhsT=ones_kk,
                    rhs=at[:, qoff:512],
                    start=(kb == 0),
                    stop=(kb == nkb - 1),
                )
                if kb == nkb - 1:
                    pending.append((hl, p, idx))

            def panel_end(hl, p):
                rb_sb = evacp.tile([128, 512], F32, tag="rb", name=f"rb{b}{hl}{p}")
                nc.vector.reciprocal_approx_fast(out=rb_sb, in_=sps.pop((hl, p)))
                nc.vector.tensor_mul(
                    y_sb[b][hl][:, p * 512:(p + 1) * 512], yps.pop((hl, p)), rb_sb
                )
                if hl == HL - 1:
                    queue_outproj(b, p)

            i = 0
            for st in flat:
                hl, p, kb = st
                if kb is None:
                    yps[(hl, p)] = psum.tile([128, 512], F32, tag="yps", bufs=2,
                                             name=f"yps{b}{hl}{p}")
                    sps[(hl, p)] = psum.tile([128, 512], F32, tag="sps", bufs=1,
                                             name=f"sps{b}{hl}{p}")
                    continue
                i += 1
                qoff = max(0, kb - 4 * p) * 128
                ps = psum.tile([128, 512], F32, tag="ps", bufs=3, name=f"sc{b}{hl}{p}{kb}")
                nc.tensor.matmul(
                    ps[:, 0:512 - qoff],
                    lhsT=k_sb[b][hl][:, kb * 128:(kb + 1) * 128],
                    rhs=q_sb[b][hl][:, p * 512 + qoff:(p + 1) * 512],
                    start=True,
                    stop=True,
                )
                if pending and pending[0][2] <= i - 1:
                    ehl, ep, _ = pending.pop(0)
                    panel_end(ehl, ep)
                if prev is not None:
                    emit_av(prev, i)
                at = attnp.tile([128, 512], BF16, tag="attn", name=f"at{b}{hl}{p}{kb}")
                nc.scalar.activation(
                    at[:, qoff:512],
                    ps[:, 0:512 - qoff],
                    mybir.ActivationFunctionType.Exp,
                    scale=SOFTMAX_SCALE,
                )
                if kb >= 4 * p:  # diagonal block: zero the k>q half
                    nc.vector.tensor_mul(
                        at[:, qoff:qoff + 128], at[:, qoff:qoff + 128], mask_sb
                    )
                state[(hl, p, kb)] = at
                prev = st
                if i % 2 == 0:
                    pump(1)
            emit_av(prev, i + 1)
            while pending:
                ehl, ep, _ = pending.pop(0)
                panel_end(ehl, ep)

        proj_batch(0)
        attn_batch(0)
        proj_batch(1)            # pumps leftover outproj(b0, p3) units
        attn_batch(1)
        pump(len(filler))        # final outproj(b1, p3) tail


def build():
    nc = bacc.Bacc("TRN2", target_bir_lowering=False, debug=False)
    xT = nc.dram_tensor("xT", [E, NTOK], BF16, kind="ExternalInput").ap()
    wqkvT = nc.dram_tensor("wqkvT", [E, 3 * HL * D], BF16, kind="ExternalInput").ap()
    w_outT = nc.dram_tensor("w_outT", [HL * D, E], BF16, kind="ExternalInput").ap()
    out = nc.dram_tensor("out", [NTOK, E], BF16, kind="ExternalOutput").ap()

    cosT, sinS = _rope_tables()
    cos_d = nc.inline_tensor(cosT.astype(ml_dtypes.bfloat16), name="cos_t").ap()
    sin_d = nc.inline_tensor(sinS.astype(ml_dtypes.bfloat16), name="sin_t").ap()
    # maskT01[k, q] = 1 where k <= q (valid), else 0 — transposed-causal
    mask = np.triu(np.ones((128, 128), np.float32)).astype(ml_dtypes.bfloat16)
    mask_d = nc.inline_tensor(mask, name="maskT01").ap()

    with tile.TileContext(nc) as tc:
        _emit(nc, tc, xT, wqkvT, w_outT, out, cos_d, sin_d, mask_d)
    nc.compile()
    return nc


def make_in_maps(x, w_qkv, w_out):
    bf = ml_dtypes.bfloat16
    x2 = np.asarray(x, np.float32).reshape(NTOK, E)
    xT = np.ascontiguousarray(x2.astype(bf).T)                      # [E, NTOK]
    w_qkv = np.asarray(w_qkv, np.float32)
    w_out = np.asarray(w_out, np.float32)
    in_maps = []
    for c in range(NCORES):
        hs = [HL * c + j for j in range(HL)]
        rows = np.concatenate(
            [w_qkv[t * E + h * D:t * E + (h + 1) * D] for t in range(3) for h in hs]
        )                                                           # [768, E]
        wqkvT = np.ascontiguousarray(rows.astype(bf).T)             # [E, 768]
        w_outT = np.ascontiguousarray(
            w_out[:, c * HL * D:(c + 1) * HL * D].astype(bf).T      # [256, E]
        )
        in_maps.append({"xT": xT, "wqkvT": wqkvT, "w_outT": w_outT})
    return in_maps


_NC = None


def kernel(x, w_qkv, w_out):
    global _NC
    if _NC is None:
        _NC = build()
    in_maps = make_in_maps(x, w_qkv, w_out)
    res = run_bass_kernel_spmd(_NC, in_maps, core_ids=list(range(NCORES)))
    total = np.zeros((NTOK, E), np.float32)
    for r in res.results:
        total += r["out"]
    return total.reshape(B, S, E)
